# revision 49
# baseline (speedup 1.0000x reference)
"""Trainium2 Bass kernel for the iterated tiny-CNN problem.

Per step (16 steps): h -> relu(b2 + w2 . tanh(b1 + conv3x3(pad(h), w1)))
with circular (wrap) padding when n == W, else constant 0.5 padding.

Strategy (data-parallel over batch, 4 images per core on 8 cores):
  - Whole per-core state (4 images of 512x512) lives in SBUF in fp16 for
    all steps; HBM traffic is load-once / store-once (fp16 both ways).
  - Per image, 4 "main" tiles of 125 primary rows + a runt tile of 12 rows
    SHARED by all 4 images (one set of matmuls covers every image's runt
    rows), stored as [128 partitions x 514 cols]:
        parts 0..124 : primary rows, part 125: halo row below,
        part 126    : constant 1.0 (bias row), part 127: halo row above,
        col slot 0  : wrap column (col 511), slots 1..512: cols 0..511,
        col slot 513: wrap column (col 0).
  - conv3x3 on the TensorEngine as banded [128->125] matmuls: vertical taps
    are tridiagonal bands (halo partitions catch the boundary rows), the 3
    horizontal taps are 3 PSUM-accumulating matmuls with rhs shifted by
    -1/0/+1 columns.  b1 rides the dj==1 band through the const-1.0 row, so
    tanh needs no per-channel bias operand and one Activation instruction
    covers both channels (4 PSUM banks, free size 2048).
  - conv2 (1x1) + bias + relu on VectorE as tensor_scalar (4x mode) +
    tensor_tensor add (2x mode) + tensor_scalar add/min (4x mode); the
    final scale w2[main] is folded into the next step's band weights so the
    whole epilogue is 3 DVE ops (the last step emits the unfolded value).
  - Halo rows refresh once per step with 6 SBUF->SBUF DMAs per image.

kernel(**inputs) takes the full unsharded inputs and returns the full
output; sharding/compile/run/gather happen inside.
"""

import numpy as np

B_FULL = 32
H = 512
W = 512
N_CORES = 8
IMGS = B_FULL // N_CORES          # images per core
NT = 4                            # main row-tiles per image
TM = 125                          # primary rows per main tile
RUNT = H - NT * TM                # rows in the shared runt tile (12)
RSP = RUNT + 2                    # partition span per image in runt tile
COLS = W + 2                      # per-tile columns incl. wrap cols
P = 128
BCOL = 128                        # lhsT column pitch for main bands
RCOL = 64                         # lhsT column pitch for runt bands
BSET = 6 * BCOL + 6 * RCOL        # one band set: 6 main + 6 runt bands

_KERNEL_CACHE = {}


def _conv2_coeffs(w2, b2):
    """u = w20*y0 + w21*y1 + b2 as u = sfin*(ratio*y_a + y_b + b2/sfin)."""
    w20 = float(w2[0, 0, 0, 0])
    w21 = float(w2[0, 1, 0, 0])
    if abs(w21) >= abs(w20):
        a_idx, ratio, sfin = 0, (w20 / w21 if w21 else 0.0), w21
    else:
        a_idx, ratio, sfin = 1, w21 / w20, w20
    return a_idx, ratio, sfin, float(b2[0])


def _build_bands(w1, b1, w2, b2):
    """Two band sets [128, 2*BSET] fp32: set 0 (raw weights, for step 0)
    and set 1 (weights scaled by sfin, for steps >= 1 whose input state
    stores v = h/sfin).  The bias b1 rides the dj==1 band via the
    const-1.0 partition (126 for main tiles, 127 for the runt tile)."""
    _, _, sfin, _ = _conv2_coeffs(w2, b2)
    out = np.zeros((128, 2 * BSET), dtype=np.float32)
    for s, wscale in ((0, 1.0), (1, sfin)):
        base = s * BSET
        wmat = w1[:, 0, :, :] * np.float32(wscale)
        for c in range(2):
            for dj in range(3):
                col0 = base + (c * 3 + dj) * BCOL
                for m in range(TM):
                    for di in range(3):
                        k = m + di - 1
                        if k == -1:
                            k = 127
                        out[k, col0 + m] = wmat[c, di, dj]
                    if dj == 1:
                        out[126, col0 + m] = np.float32(b1[c])
        for c in range(2):
            for dj in range(3):
                col0 = base + 6 * BCOL + (c * 3 + dj) * RCOL
                for i in range(IMGS):
                    for j in range(1, RUNT + 1):
                        m = i * RSP + j
                        for di in range(3):
                            out[m + di - 1, col0 + m] = wmat[c, di, dj]
                        if dj == 1:
                            out[127, col0 + m] = np.float32(b1[c])
    return out


def _split_waits(nc, max_inline=1):
    """The walrus build here allows only one sync-wait per instruction;
    hoist extra waits into preceding same-engine NoOps (what raw bass's
    explicit wait_ge does)."""
    import concourse.mybir as mybir
    total = 0
    for fn in nc.m.functions:
        for blk in fn.blocks:
            insts = list(blk.instructions)
            new = []
            for ins in insts:
                si = ins.sync_info
                ow = list(si.on_wait) if si is not None else []
                if len(ow) > max_inline:
                    for w in ow[:-max_inline]:
                        nop = mybir.InstNoOp(
                            name=nc.get_next_instruction_name(),
                            engine=ins.engine,
                            ins=[], outs=[],
                            sync_info=mybir.SyncInfo(on_wait=[w],
                                                     on_update=[]),
                        )
                        new.append(nop)
                        total += 1
                    ins.sync_info = mybir.SyncInfo(
                        on_wait=ow[-max_inline:],
                        on_update=list(si.on_update))
                new.append(ins)
            blk.instructions = new
    return total


def _build_nc(steps, wrap, w1, b1, w2, b2, dt16=True):
    import concourse.bass as bass
    import concourse.mybir as mybir
    from concourse.tile import TileContext

    dt = mybir.dt
    DT = dt.float16
    Alu = mybir.AluOpType
    Act = mybir.ActivationFunctionType

    a_idx, ratio, sfin, b2f = _conv2_coeffs(w2, b2)
    b2p = b2f / sfin if sfin else 0.0
    clip_op = Alu.max if sfin >= 0 else Alu.min

    def rap(base, extra, dims):
        """Raw AP into `base` (an AP) at base.offset + extra with explicit
        [step, count] dims; dims[0] is the partition dim."""
        return bass.AP(base.tensor, base.offset + extra, dims)

    nc = bass.Bass()
    xs = nc.dram_tensor("xs", [IMGS, H, W], DT, kind="ExternalInput")
    bands = nc.dram_tensor("bands", [128, 2 * BSET], DT,
                           kind="ExternalInput")
    out = nc.dram_tensor("out", [IMGS, H, W], DT, kind="ExternalOutput")
    # row 0: 1.0 (bias/const rows), row 1: 0.5 (constant-pad halos),
    # row 2: 0.5/sfin (pad value in the sfin-folded units of steps >= 1)
    cst = nc.dram_tensor("cst", [3, NT * COLS], DT, kind="ExternalInput")

    with TileContext(nc) as tc:
        with (
            tc.tile_pool(name="state", bufs=1) as state_pool,
            tc.tile_pool(name="const", bufs=1) as const_pool,
            tc.tile_pool(name="psum", bufs=4, space="PSUM") as psum_pool,
            tc.tile_pool(name="scratch", bufs=3) as scratch_pool,
        ):
            band_t = const_pool.tile([128, 2 * BSET], DT, tag="bands")
            # Main set-0 bands first: they gate the very first matmul.
            # Issued on the Pool SWDGE queue so they load in parallel with
            # image 0's HWDGE chain.  (set 1 follows after image 0's load)
            nc.gpsimd.dma_start(band_t[:, 0:6 * BCOL], bands[:, 0:6 * BCOL])

            state = []
            for i in range(IMGS):
                st = state_pool.tile([P, NT * COLS], DT,
                                     tag=f"state{i}", name=f"state{i}")
                state.append(st)
            rt = state_pool.tile([P, COLS], DT, tag="runt", name="runt")
            pitch = [st.ap[0][0] for st in state]
            rpitch = rt.ap[0][0]

            def lhsT(s, c, dj):
                col0 = (1 if s else 0) * BSET + (c * 3 + dj) * BCOL
                return band_t[:, col0:col0 + TM]

            def lhsT_runt(s, c, dj):
                col0 = ((1 if s else 0) * BSET + 6 * BCOL
                        + (c * 3 + dj) * RCOL)
                return band_t[:, col0:col0 + IMGS * RSP]

            # ---- initial load (xs is pre-cast fp16 on the host) ----
            # Zero the runt tile first: partitions 56..126 stay zero and
            # feed the matmul K dim (garbage there would NaN PSUM).
            nc.gpsimd.memset(rt[:, :], 0.0)
            nc.sync.dma_start(rt[127:128, :], cst[0:1, 0:COLS])

            def emit_load(i, skip_main=False, skip_runt=False):
                st = state[i]
                xbase = xs[i, 0:1, :]
                # image 0 gates the first matmuls: its DMAs go on the fast
                # HWDGE queue; the other images' small loads ride the idle
                # Pool SWDGE queue to keep HWDGE free for step-0 halos.
                dma = nc.sync.dma_start if i == 0 else nc.gpsimd.dma_start
                if i == 0 and wrap:
                    # pair01's critical deps first: tiles 0,1 + their halos
                    nc.sync.dma_start(
                        rap(st, 1, [[pitch[i], TM + 1], [COLS, 2], [1, W]]),
                        rap(xbase, 0, [[W, TM + 1], [TM * W, 2], [1, W]]))
                    nc.sync.dma_start(st[127:128, 1:1 + W],
                                      xs[i, H - 1: H, :])
                    nc.sync.dma_start(
                        rap(st, 127 * pitch[i] + COLS + 1,
                            [[pitch[i], 1], [COLS, 1], [1, W]]),
                        rap(xbase, (TM - 1) * W,
                            [[H * W, 1], [TM * W, 1], [1, W]]))
                    nc.gpsimd.dma_start(state[i][126:127, :], cst[0:1, :])
                    src01 = rap(st, 1, [[pitch[i], P], [COLS, 2], [511, 2]])
                    dst01 = rap(st, 513, [[pitch[i], P], [COLS, 2],
                                          [-513, 2]])
                    nc.vector.tensor_copy(dst01, src01)
                    # prefetch image 1's main tiles ahead of image 0's
                    # tiles 2,3 so pair01(1) isn't load-gated in step 0
                    st1 = state[1]
                    xb1 = xs[1, 0:1, :]
                    nc.sync.dma_start(
                        rap(st1, 1, [[pitch[1], TM + 1], [COLS, NT],
                                     [1, W]]),
                        rap(xb1, 0, [[W, TM + 1], [TM * W, NT], [1, W]]))
                    # rest of image 0: tiles 2,3 + their halo-above rows
                    nc.sync.dma_start(
                        rap(st, 2 * COLS + 1,
                            [[pitch[i], TM + 1], [COLS, 2], [1, W]]),
                        rap(xbase, 2 * TM * W,
                            [[W, TM + 1], [TM * W, 2], [1, W]]))
                    nc.sync.dma_start(
                        rap(st, 127 * pitch[i] + 2 * COLS + 1,
                            [[pitch[i], 1], [COLS, 2], [1, W]]),
                        rap(xbase, (2 * TM - 1) * W,
                            [[H * W, 1], [TM * W, 2], [1, W]]))
                    # the shared runt round reads every image's runt rows
                    # early in step 0: load them all now on HWDGE (incl.
                    # image 0's, which must precede emit_wrap_cols_runt)
                    for j in range(IMGS):
                        nc.sync.dma_start(
                            rt[j * RSP: j * RSP + 1 + RUNT, 1:1 + W],
                            xs[j, NT * TM - 1: H, :])
                        nc.sync.dma_start(
                            rt[j * RSP + RUNT + 1: j * RSP + RUNT + 2,
                               1:1 + W],
                            xs[j, 0:1, :])
                    emit_wrap_cols_runt()
                else:
                    if not skip_main:
                        # primary rows + halo-below of all 4 tiles, one DMA
                        nc.sync.dma_start(
                            rap(st, 1, [[pitch[i], TM + 1], [COLS, NT],
                                        [1, W]]),
                            rap(xbase, 0,
                                [[W, TM + 1], [TM * W, NT], [1, W]]))
                    # halo-above rows for tiles 1..3 straight from HBM
                    dma(rap(st, 127 * pitch[i] + COLS + 1,
                            [[pitch[i], 1], [COLS, NT - 1], [1, W]]),
                        rap(xbase, (TM - 1) * W,
                            [[H * W, 1], [TM * W, NT - 1], [1, W]]))
                if not skip_runt:
                    # runt primary rows + halo-above (rows 499..511)
                    dma(rt[i * RSP: i * RSP + 1 + RUNT, 1:1 + W],
                        xs[i, NT * TM - 1: H, :])
                if wrap:
                    if i != 0:
                        # tile0 halo-above <- row 511
                        dma(st[127:128, 1:1 + W], xs[i, H - 1: H, :])
                    if not skip_runt:
                        # runt halo-below <- row 0
                        dma(rt[i * RSP + RUNT + 1: i * RSP + RUNT + 2,
                               1:1 + W],
                            xs[i, 0:1, :])
                if not (i == 0 and wrap):
                    dma(state[i][126:127, :], cst[0:1, :])

            def emit_wrap_cols(i, tiles):
                # slot0 <- col 511 (slot 512), slot513 <- col 0 (slot 1)
                st = state[i]
                nt = len(tiles)
                t0 = tiles[0]
                if wrap:
                    src = rap(st, t0 * COLS + 1,
                              [[pitch[i], TM], [COLS, nt], [511, 2]])
                    dst = rap(st, t0 * COLS + 513,
                              [[pitch[i], TM], [COLS, nt], [-513, 2]])
                    nc.vector.tensor_copy(dst, src)
                else:
                    for t in tiles:
                        nc.vector.memset(
                            st[:, t * COLS: t * COLS + 1], 0.5)
                        nc.vector.memset(
                            st[:, t * COLS + 513: t * COLS + 514], 0.5)

            def emit_wrap_cols_runt():
                if wrap:
                    src = rap(rt, 1, [[rpitch, IMGS * RSP], [511, 2]])
                    dst = rap(rt, 513, [[rpitch, IMGS * RSP], [-513, 2]])
                    nc.vector.tensor_copy(dst, src)
                else:
                    nc.vector.memset(rt[:, 0:1], 0.5)
                    nc.vector.memset(rt[:, 513:514], 0.5)

            def emit_halos_main(i):
                """Halo refreshes staying within image i's main tiles."""
                st = state[i]
                # p127 of tiles 1..3 <- p124 of tiles 0..2
                nc.gpsimd.dma_start(st[127:128, COLS:NT * COLS],
                                    st[124:125, 0:(NT - 1) * COLS])
                # p125 of tiles 0..2 <- p0 of tiles 1..3
                nc.gpsimd.dma_start(st[125:126, 0:(NT - 1) * COLS],
                                    st[0:1, COLS:NT * COLS])

            def emit_halos_rt(i):
                """Runt-tile halo refreshes (write rt: must trail the
                shared runt round of the current step)."""
                st = state[i]
                # runt part i*RSP (row 499) <- tile 3 p124
                nc.sync.dma_start(rt[i * RSP: i * RSP + 1, :],
                                  st[124:125, (NT - 1) * COLS:NT * COLS])
                if wrap:
                    # runt part i*RSP+RUNT+1 (row "512" = row 0) <- tile0 p0
                    nc.sync.dma_start(
                        rt[i * RSP + RUNT + 1: i * RSP + RUNT + 2, :],
                        st[0:1, 0:COLS])
                else:
                    # the runt epilogue clobbered the bottom pad row
                    nc.sync.dma_start(
                        rt[i * RSP + RUNT + 1: i * RSP + RUNT + 2, 1:1 + W],
                        cst[2:3, 0:W])

            def emit_halos_b(i):
                """Halo refreshes sourced from the runt-tile write."""
                st = state[i]
                # p125 of tile 3 <- runt row 500 (rt part i*RSP+1)
                nc.sync.dma_start(st[125:126, (NT - 1) * COLS:NT * COLS],
                                  rt[i * RSP + 1: i * RSP + 2, :])
                if wrap:
                    # p127 of tile 0 <- row 511 (rt part i*RSP+RUNT)
                    nc.sync.dma_start(
                        st[127:128, 0:COLS],
                        rt[i * RSP + RUNT: i * RSP + RUNT + 1, :])

            for i in range(IMGS):
                emit_load(i, skip_main=(wrap and i == 1),
                          skip_runt=wrap)
                if i == 0:
                    nc.sync.dma_start(band_t[:, 6 * BCOL:],
                                      bands[:, 6 * BCOL:])
                # wrap-col slots for every loaded partition (incl. halo and
                # const rows; the const row maps 1.0 -> 1.0).
                if wrap:
                    t0 = 2 if i == 0 else 0
                    src = rap(state[i], t0 * COLS + 1,
                              [[pitch[i], P], [COLS, NT - t0], [511, 2]])
                    dst = rap(state[i], t0 * COLS + 513,
                              [[pitch[i], P], [COLS, NT - t0], [-513, 2]])
                    nc.vector.tensor_copy(dst, src)
                else:
                    for t in range(NT):
                        nc.vector.memset(
                            state[i][:, t * COLS: t * COLS + 1], 0.5)
                        nc.vector.memset(
                            state[i][:, t * COLS + 513: t * COLS + 514], 0.5)
                    nc.vector.memset(rt[:, 0:1], 0.5)
                    nc.vector.memset(rt[:, 513:514], 0.5)
                    nc.sync.dma_start(state[i][127:128, 1:1 + W],
                                      cst[1:2, 0:W])
                    nc.sync.dma_start(
                        rt[i * RSP + RUNT + 1: i * RSP + RUNT + 2, 1:1 + W],
                        cst[1:2, 0:W])

            y_off = [0, 2 * W]  # per-channel column offset into y tiles
            a_off = y_off[a_idx]
            b_off = y_off[1 - a_idx]

            def epilogue(pss, m, fd, ao, bo, dstp, last):
                """tanh + conv2 + bias (+relu) from per-channel PSUM tiles
                pss (each [0:m, free 0:fd]) into dstp; y gets the two
                channel blocks at free offsets ao/bo."""
                yt = scratch_pool.tile([P, 4 * W], DT, tag="y", name="y")
                for c, ps in enumerate(pss):
                    pp = ps.ap[0][0]
                    nc.scalar.activation(
                        yt[0:m, c * fd: (c + 1) * fd],
                        rap(ps, 0, [[pp, m], [1, fd]]),
                        Act.Tanh)
                ta = scratch_pool.tile([P, 2 * W], DT, tag="ta", name="ta")
                nc.vector.tensor_scalar_mul(
                    ta[0:m, 0:fd], yt[0:m, ao:ao + fd], ratio)
                tb = scratch_pool.tile([P, 2 * W], DT, tag="tb", name="tb")
                nc.vector.tensor_tensor(
                    tb[0:m, 0:fd], ta[0:m, 0:fd],
                    yt[0:m, bo:bo + fd], Alu.add)
                if not last:
                    nc.vector.tensor_scalar(
                        dstp, tb[0:m, 0:fd], b2p, 0.0, Alu.add, clip_op)
                else:
                    tu = scratch_pool.tile([P, 2 * W], DT,
                                           tag="tu", name="tu")
                    nc.vector.tensor_scalar(
                        tu[0:m, 0:fd], tb[0:m, 0:fd], sfin, b2f,
                        Alu.mult, Alu.add)
                    nc.vector.tensor_scalar_max(dstp, tu[0:m, 0:fd], 0.0)

            def emit_pairs(i, bs, last, store=False, fine=False):
                st = state[i]
                rounds = (((0,), (1,), (2,), (3,)) if fine
                          else ((0, 1), (2, 3)))
                for tp in rounds:
                    t0 = tp[0]
                    nt = len(tp)
                    pss = []
                    for c in range(2):
                        ps = psum_pool.tile([P, 2, W], dt.float32,
                                            tag="ps", name=f"ps{c}")
                        pss.append(ps)
                        for j, t in enumerate(tp):
                            for dj in range(3):
                                rhs = st[0:P,
                                         t * COLS + dj:t * COLS + dj + W]
                                nc.tensor.matmul(
                                    ps[0:TM, j, :],
                                    lhsT(bs, c, dj), rhs,
                                    start=(dj == 0), stop=(dj == 2))
                    dstp = rap(st, t0 * COLS + 1,
                               [[pitch[i], TM], [COLS, nt], [1, W]])
                    epilogue(pss, TM, nt * W, a_off * nt // 2,
                             b_off * nt // 2, dstp, last)
                    if wrap and not last:
                        emit_wrap_cols(i, tp)
                    if store:
                        obase = out[i, 0:1, :]
                        nc.sync.dma_start(
                            rap(obase, tp[0] * TM * W,
                                [[W, TM], [TM * W, len(tp)], [1, W]]),
                            rap(st, tp[0] * COLS + 1,
                                [[pitch[i], TM], [COLS, len(tp)], [1, W]]))
                if not last:
                    emit_halos_main(i)

            def emit_runt(bs, last):
                # shared runt round (all 4 images in one matmul set)
                M = IMGS * RSP
                pssr = []
                for c in range(2):
                    psr = psum_pool.tile([P, 2, W], dt.float32,
                                         tag="ps", name=f"psr{c}")
                    pssr.append(psr)
                    for dj in range(3):
                        rhs = rt[0:P, dj:dj + W]
                        nc.tensor.matmul(psr[0:M, 0, :],
                                         lhsT_runt(bs, c, dj), rhs,
                                         start=(dj == 0), stop=(dj == 2))
                epilogue(pssr, M, W, a_off // 2, b_off // 2,
                         rt[0:M, 1:1 + W], last)
                if wrap and not last:
                    emit_wrap_cols_runt()

            # ---- steps ----
            # Round order: images 0,1 -> shared runt -> images 2,3, with
            # halo DMAs issued as soon as their sources are written AND all
            # this-step readers of their destinations have run (the runt
            # round reads every image's rt halo slots, so rt-touching DMAs
            # trail it; tile0-p127/tile3-p125 DMAs trail that image's own
            # pair rounds).  This keeps every refresh ~half a step ahead of
            # its first consumer in the next step, so nothing stalls.
            def emit_store(i):
                for t in range(NT):
                    nc.sync.dma_start(
                        out[i, t * TM: (t + 1) * TM, :],
                        state[i][0:TM, t * COLS + 1: t * COLS + 1 + W])

            def emit_store_runt():
                for i in range(IMGS):
                    nc.sync.dma_start(
                        out[i, NT * TM: H, :],
                        rt[i * RSP + 1: i * RSP + 1 + RUNT, 1:1 + W])

            def emit_pad_rescale():
                # Non-wrap only: from step 1 on the state stores v = h/sfin,
                # so every constant-pad cell must hold 0.5/sfin, not 0.5.
                pv = 0.5 / sfin if sfin else 0.0
                for i in range(IMGS):
                    for t in range(NT):
                        nc.vector.memset(
                            state[i][:, t * COLS: t * COLS + 1], pv)
                        nc.vector.memset(
                            state[i][:, t * COLS + 513: t * COLS + 514], pv)
                    nc.sync.dma_start(state[i][127:128, 1:1 + W],
                                      cst[2:3, 0:W])
                nc.vector.memset(rt[:, 0:1], pv)
                nc.vector.memset(rt[:, 513:514], pv)
                for i in range(IMGS):
                    nc.sync.dma_start(
                        rt[i * RSP + RUNT + 1: i * RSP + RUNT + 2, 1:1 + W],
                        cst[2:3, 0:W])

            for s in range(steps):
                bs = 1 if s else 0
                last = (s == steps - 1)
                if s == 1 and not wrap:
                    emit_pad_rescale()
                for i in (0, 1):
                    emit_pairs(i, bs, last, store=last)
                emit_runt(bs, last)
                if last:
                    emit_store_runt()
                else:
                    for i in (0, 1):
                        emit_halos_rt(i)
                        emit_halos_b(i)
                for i in (2, 3):
                    emit_pairs(i, bs, last, store=last,
                               fine=(last and i == 3))
                    if not last:
                        emit_halos_rt(i)
                        emit_halos_b(i)

            if steps == 0:
                for i in range(IMGS):
                    emit_store(i)
                emit_store_runt()
    _split_waits(nc)
    return nc


class _Runner:
    """Persistent jitted shard_map runner for a built Bass module
    (mirrors concourse.bass2jax.run_bass_via_pjrt, but reusable across
    calls and usable with device-resident inputs for timing)."""

    def __init__(self, nc):
        import jax
        import numpy as _np
        import concourse.mybir as mybir
        from jax.sharding import Mesh, PartitionSpec
        from jax.experimental.shard_map import shard_map
        from concourse import bass2jax

        bass2jax.install_neuronx_cc_hook()
        assert nc.dbg_addr is None

        partition_name = (nc.partition_id_tensor.name
                          if nc.partition_id_tensor else None)
        in_names, out_names, out_avals = [], [], []
        for alloc in nc.m.functions[0].allocations:
            if not isinstance(alloc, mybir.MemoryLocationSet):
                continue
            name = alloc.memorylocations[0].name
            if alloc.kind == "ExternalInput":
                if name != partition_name:
                    in_names.append(name)
            elif alloc.kind == "ExternalOutput":
                out_names.append(name)
                out_avals.append(jax.core.ShapedArray(
                    tuple(alloc.tensor_shape), mybir.dt.np(alloc.dtype)))
        self.in_names = in_names
        self.out_names = out_names
        self.out_avals = out_avals
        all_in_names = in_names + out_names
        if partition_name is not None:
            all_in_names = all_in_names + [partition_name]

        def _body(*args):
            operands = list(args)
            if partition_name is not None:
                operands.append(bass2jax.partition_id_tensor())
            outs = bass2jax._bass_exec_p.bind(
                *operands,
                out_avals=tuple(out_avals),
                in_names=tuple(all_in_names),
                out_names=tuple(out_names),
                lowering_input_output_aliases=(),
                sim_require_finite=True,
                sim_require_nnan=True,
                nc=nc,
            )
            return tuple(outs)

        devices = jax.devices()[:N_CORES]
        self.mesh = Mesh(_np.asarray(devices), ("core",))
        n_all = len(in_names) + len(out_names)
        self.fn = jax.jit(
            shard_map(_body, mesh=self.mesh,
                      in_specs=(PartitionSpec("core"),) * n_all,
                      out_specs=(PartitionSpec("core"),) * len(out_names),
                      check_rep=False),
            keep_unused=True,
        )

    def concat_inputs(self, in_maps):
        """Per-core in_maps -> global concat arrays (+ zero out bufs)."""
        arrs = []
        for name in self.in_names:
            arrs.append(np.concatenate(
                [np.asarray(m[name]) for m in in_maps], axis=0))
        for av in self.out_avals:
            arrs.append(np.zeros((N_CORES * av.shape[0],) + av.shape[1:],
                                 av.dtype))
        return arrs

    def __call__(self, *arrs):
        return self.fn(*arrs)

    def run(self, in_maps):
        out_arrs = self.fn(*self.concat_inputs(in_maps))
        res = []
        for c in range(N_CORES):
            res.append({
                name: np.asarray(out_arrs[i]).reshape(
                    (N_CORES,) + self.out_avals[i].shape)[c]
                for i, name in enumerate(self.out_names)})
        return res


def _get_runner(key, steps, wrap, w1, b1, w2, b2, dt16):
    if key not in _KERNEL_CACHE:
        nc = _build_nc(steps, wrap, w1, b1, w2, b2, dt16=dt16)
        _KERNEL_CACHE[key] = _Runner(nc)
    return _KERNEL_CACHE[key]


def _prep(x, w1, b1, w2, b2, steps, n, dt16=True):
    x = np.asarray(x)
    w1 = np.asarray(w1, dtype=np.float32)
    b1 = np.asarray(b1, dtype=np.float32)
    w2 = np.asarray(w2, dtype=np.float32)
    b2 = np.asarray(b2, dtype=np.float32)
    steps = int(steps)
    n = int(n)
    wrap = (n == W)
    xf = np.ascontiguousarray(
        x.reshape(B_FULL, H, W).astype(np.float16))
    bands = _build_bands(w1, b1, w2, b2).astype(np.float16)
    key = (steps, wrap, dt16, w1.tobytes(), b1.tobytes(), w2.tobytes(),
           b2.tobytes())
    runner = _get_runner(key, steps, wrap, w1, b1, w2, b2, dt16)
    _, _, sfin_, _ = _conv2_coeffs(w2, b2)
    cstv = np.zeros((3, NT * COLS), dtype=np.float16)
    cstv[0] = 1.0
    cstv[1] = 0.5
    cstv[2] = 0.5 / sfin_ if sfin_ else 0.0
    in_maps = [{"xs": xf[c * IMGS:(c + 1) * IMGS], "bands": bands,
                "cst": cstv}
               for c in range(N_CORES)]
    return runner, in_maps


def kernel(x, w1, b1, w2, b2, steps, n):
    in_dtype = np.asarray(x).dtype
    runner, in_maps = _prep(x, w1, b1, w2, b2, steps, n)
    res = runner.run(in_maps)
    full = np.concatenate([r["out"] for r in res], axis=0)
    full = full.reshape(B_FULL, 1, H, W)
    return full.astype(in_dtype, copy=False)
